# revision 7
# baseline (speedup 1.0000x reference)
"""Trainium2 Bass kernel for nn_Attention (B=2, S=2048, H=2048, NH=16, HD=128).

Sharding: 2-way batch DP x 4-way head TP -> 8 NeuronCores.
Core c = b*4 + hq handles batch b, heads [4*hq, 4*hq+4).

Per-core pipeline (all fp32 storage, matmuls optionally float32r mode):
  Phase A: Q/K/V projections from resident x^T; RoPE fused into the PSUM
           evacuation of Q/K; q^T/k^T/v spilled to DRAM scratch.
  Phase B: per (head, q-chunk of 512): scores computed TRANSPOSED
           (S^T[k,q] = K^T stationary x Q^T moving), additive bias tiles only
           where the host-side classification says they are needed, exp on
           ACT (PSUM->SBUF), denominator via ones-matmul PSUM accumulation,
           PV accumulation with V stationary, normalization by broadcast
           reciprocal fused into the PV PSUM evacuation.
  Phase C: partial O-projection (contraction over this core's 512 attention
           features) -> out_partial [2048, 2048].
Host sums the 4 head-group partials per batch.

Causal masking is exploited structurally: host classifies each
(q-chunk 512, k-tile 128) tile of (attn_bias + masks) as SKIP (all <= -1e8),
ZERO (all == 0) or GENERAL (bias data streamed + added). Fully-masked score
entries in GENERAL tiles underflow exp() to exactly 0.0, matching the
reference's softmax on -1e9-masked logits. Softmax max-subtraction is skipped:
logits here are O(10) so exp() cannot overflow, and the host verifies every
row has at least one live (not strongly negative) tile.
"""
import math
import sys

sys.path.insert(0, '/opt/trn_rl_repo')

import numpy as np

B, S, H, NH, HD = 2, 2048, 2048, 16, 128
N_CORES = 8
HPC = 4               # heads per core
QC = 512              # q-chunk (matmul moving free dim)
KT = 128              # k-tile (PE contraction dim)
NQ = S // QC          # 4
NKT = S // KT         # 16
DPC = HPC * HD        # 512 features per core

SKIP, ZERO, GEN = 0, 1, 2

# matmul dtype knobs: float32r streams fp32 through the PE in single-pass
# mode (4x faster at free dim >= 256) at reduced internal precision.
USE_F32R = True
DEBUG_DUMP = False

LAST_EXEC_TIME_NS = None
LAST_RESULTS = None


def _classify(combined):
    """combined: [B, S, S] additive bias (attn_bias + masks), b-th batch.
    Returns cls[NQ][NKT] merged over batches, and per-batch GEN tile data."""
    cls = np.full((NQ, NKT), ZERO, np.int32)
    per_b = np.zeros((B, NQ, NKT), np.int32)
    for j in range(NQ):
        for i in range(NKT):
            for b in range(B):
                t = combined[b, j * QC:(j + 1) * QC, i * KT:(i + 1) * KT]
                if t.max() <= -1e8:
                    per_b[b, j, i] = SKIP
                elif not t.any():
                    per_b[b, j, i] = ZERO
                else:
                    per_b[b, j, i] = GEN
    for j in range(NQ):
        for i in range(NKT):
            kinds = set(per_b[:, j, i])
            if kinds == {SKIP}:
                cls[j, i] = SKIP
            elif kinds == {ZERO}:
                cls[j, i] = ZERO
            else:
                cls[j, i] = GEN
    return cls


def _build_and_run(in_maps, cls, n_gen):
    import concourse.bacc as bacc
    import concourse.mybir as mybir
    import concourse.tile as tile
    from concourse import bass_utils

    try:
        sys.path.insert(0, '/root/problem')
        import axon_profile_patch
        axon_profile_patch.install()
    except Exception:
        pass

    F32 = mybir.dt.float32
    F32R = mybir.dt.float32r
    EXP = mybir.ActivationFunctionType.Exp
    ADD = mybir.AluOpType.add
    MULT = mybir.AluOpType.mult

    MMDT = F32R if USE_F32R else F32

    nc = bacc.Bacc("TRN2", target_bir_lowering=False, debug=False,
                   num_devices=N_CORES)

    xT_d = nc.dram_tensor("xT", [NKT, KT, S], MMDT, kind="ExternalInput").ap()
    wq_d = nc.dram_tensor("wq", [HPC, NKT, KT, HD], MMDT, kind="ExternalInput").ap()
    wk_d = nc.dram_tensor("wk", [HPC, NKT, KT, HD], MMDT, kind="ExternalInput").ap()
    wv_d = nc.dram_tensor("wv", [NKT, KT, DPC], MMDT, kind="ExternalInput").ap()
    wo_d = nc.dram_tensor("woT", [HPC, KT, S], MMDT, kind="ExternalInput").ap()
    cq_d = nc.dram_tensor("cos_q", [HD, S], F32, kind="ExternalInput").ap()
    sq_d = nc.dram_tensor("sinm_q", [HD, S], F32, kind="ExternalInput").ap()
    ck_d = nc.dram_tensor("cos_k", [HD, S], F32, kind="ExternalInput").ap()
    sk_d = nc.dram_tensor("sinm_k", [HD, S], F32, kind="ExternalInput").ap()
    if n_gen:
        bg_d = nc.dram_tensor("bias_gen", [n_gen, KT, QC], F32,
                              kind="ExternalInput").ap()
    ones_d = nc.dram_tensor("ones", [KT, 1], MMDT, kind="ExternalInput").ap()
    out_d = nc.dram_tensor("out", [S, S], F32, kind="ExternalOutput").ap()
    if DEBUG_DUMP:
        dbg_q = nc.dram_tensor("dbg_q", [HPC, HD, S], MMDT, kind="ExternalOutput").ap()
        dbg_k = nc.dram_tensor("dbg_k", [HPC, HD, S], MMDT, kind="ExternalOutput").ap()
        dbg_v = nc.dram_tensor("dbg_v", [NKT, KT, DPC], MMDT, kind="ExternalOutput").ap()
        dbg_attn = nc.dram_tensor("dbg_attn", [HD, HPC, S], MMDT, kind="ExternalOutput").ap()

    with tile.TileContext(nc) as tc:
        with tc.tile_pool(name="dram", bufs=1, space="DRAM") as dpool:
            qts = [dpool.tile([HD, S], MMDT, tag=f"qts{h}", name=f"qts{h}") for h in range(HPC)]
            kts_d = [dpool.tile([HD, S], MMDT, tag=f"kts{h}", name=f"kts{h}") for h in range(HPC)]
            vts = dpool.tile([NKT, KT, DPC], MMDT, tag="vts", name="vts")

            with tc.tile_pool(name="small", bufs=1) as spool:
                ones_col = spool.tile([KT, 1], MMDT, tag="ones_col", name="ones_col")
                nc.sync.dma_start(ones_col[:], ones_d[:])
                ones_row = spool.tile([1, KT], F32, tag="ones_row", name="ones_row")
                nc.vector.memset(ones_row[:], 1.0)

                # ---------------- Phase A: projections + RoPE ----------
                with tc.tile_pool(name="xp", bufs=1) as xp:
                    x_sb = [xp.tile([KT, S], MMDT, tag=f"x{kt}", name=f"x{kt}")
                            for kt in range(NKT)]
                    for kt in range(NKT):
                        nc.sync.dma_start(x_sb[kt][:], xT_d[kt])

                    # Q then K (rope tables swapped between the two passes)
                    for (w_d, c_d, s_d, dsts) in ((wq_d, cq_d, sq_d, qts),
                                                  (wk_d, ck_d, sk_d, kts_d)):
                        with tc.tile_pool(name="qk", bufs=2) as qkp, \
                             tc.tile_pool(name="rope", bufs=1) as rp, \
                             tc.tile_pool(name="qkps", bufs=2, space="PSUM") as pp:
                            cos_sb = rp.tile([HD, S], F32, tag="cos", name="cos")
                            sin_sb = rp.tile([HD, S], F32, tag="sin", name="sin")
                            nc.sync.dma_start(cos_sb[:], c_d[:])
                            nc.sync.dma_start(sin_sb[:], s_d[:])
                            for h in range(HPC):
                                w_sb = qkp.tile([KT, NKT, HD], MMDT, tag="w", name="w")
                                for kt in range(NKT):
                                    nc.sync.dma_start(w_sb[:, kt, :], w_d[h, kt])
                                for sc in range(NQ):
                                    ps = pp.tile([KT, QC], F32, tag="ps", name="ps")
                                    for kt in range(NKT):
                                        nc.tensor.matmul(
                                            ps[:],
                                            lhsT=(w_sb[:, kt, :]),
                                            rhs=(x_sb[kt][:, sc * QC:(sc + 1) * QC]),
                                            start=(kt == 0), stop=(kt == NKT - 1))
                                    st = qkp.tile([KT, QC], F32, tag="st", name="st")
                                    sw = qkp.tile([KT, QC], F32, tag="sw", name="sw")
                                    str_ = qkp.tile([KT, QC], MMDT, tag="str", name="str")
                                    csl = slice(sc * QC, (sc + 1) * QC)
                                    # rotate-half via partition-offset reads
                                    nc.vector.tensor_mul(
                                        sw[0:64, :], ps[64:128, :], sin_sb[0:64, csl])
                                    nc.vector.tensor_mul(
                                        sw[64:128, :], ps[0:64, :], sin_sb[64:128, csl])
                                    nc.vector.tensor_mul(st[:], ps[:], cos_sb[:, csl])
                                    nc.vector.tensor_add(str_[:], st[:], sw[:])
                                    nc.sync.dma_start(dsts[h][:, csl], str_[:])
                                    if DEBUG_DUMP:
                                        dbg = dbg_q if dsts is qts else dbg_k
                                        nc.sync.dma_start(dbg[h][:, csl], str_[:])

                    # V projection: x^T chunks as stationary, wv as moving
                    with tc.tile_pool(name="vw", bufs=2) as vwp, \
                         tc.tile_pool(name="vst", bufs=3) as vsp, \
                         tc.tile_pool(name="vps", bufs=1, space="PSUM") as vpp:
                        for mtg in range(2):
                            pss = [vpp.tile([KT, DPC], F32, tag=f"vps{m}", name=f"vps{m}")
                                   for m in range(8)]
                            for kt in range(NKT):
                                wv_sb = vwp.tile([KT, DPC], MMDT, tag="wv", name="wv")
                                nc.sync.dma_start(wv_sb[:], wv_d[kt])
                                for m in range(8):
                                    mt = mtg * 8 + m
                                    nc.tensor.matmul(
                                        pss[m][:],
                                        lhsT=(x_sb[kt][:, mt * KT:(mt + 1) * KT]),
                                        rhs=(wv_sb[:]),
                                        start=(kt == 0), stop=(kt == NKT - 1))
                            for m in range(8):
                                mt = mtg * 8 + m
                                vst = vsp.tile([KT, DPC], MMDT, tag="vst", name="vst")
                                nc.scalar.copy(vst[:], pss[m][:])
                                nc.sync.dma_start(vts[mt], vst[:])

                # ---------------- Phase B: attention --------------------
                with tc.tile_pool(name="attn", bufs=1) as ap_pool:
                    attn_sb = ap_pool.tile([HD, HPC, S], MMDT, tag="attn", name="attn")

                    with tc.tile_pool(name="bt", bufs=1) as btp, \
                         tc.tile_pool(name="kv", bufs=2) as kvp, \
                         tc.tile_pool(name="pt", bufs=4) as ptp, \
                         tc.tile_pool(name="sps", bufs=2, space="PSUM") as spp, \
                         tc.tile_pool(name="ops", bufs=2, space="PSUM") as opp, \
                         tc.tile_pool(name="dps", bufs=2, space="PSUM") as dpp, \
                         tc.tile_pool(name="bps", bufs=2, space="PSUM") as bpp:

                        bias_sb = {}
                        gi = 0
                        for j in range(NQ):
                            for i in range(NKT):
                                if cls[j][i] == GEN:
                                    t = btp.tile([KT, QC], F32, tag=f"bias{gi}", name=f"bias{gi}")
                                    nc.sync.dma_start(t[:], bg_d[gi])
                                    bias_sb[(j, i)] = t
                                    gi += 1

                        for h in range(HPC):
                            k_sb = kvp.tile([HD, S], MMDT, tag="k", name="k")
                            nc.sync.dma_start(k_sb[:], kts_d[h][:])
                            v_sb = kvp.tile([KT, NKT, HD], MMDT, tag="v", name="v")
                            for kt in range(NKT):
                                nc.sync.dma_start(
                                    v_sb[:, kt, :],
                                    vts[kt][:, h * HD:(h + 1) * HD])
                            for j in range(NQ):
                                live = [i for i in range(NKT) if cls[j][i] != SKIP]
                                q_sb = kvp.tile([HD, QC], MMDT, tag="q", name="q")
                                nc.sync.dma_start(
                                    q_sb[:], qts[h][:, j * QC:(j + 1) * QC])
                                ps_o = opp.tile([HD, QC], F32, tag="o", name="o")
                                ps_den = dpp.tile([1, QC], F32, tag="den", name="den")
                                for idx, i in enumerate(live):
                                    ps_s = spp.tile([KT, QC], F32, tag="s", name="s")
                                    nc.tensor.matmul(
                                        ps_s[:],
                                        lhsT=(k_sb[:, i * KT:(i + 1) * KT]),
                                        rhs=(q_sb[:]),
                                        start=True, stop=True)
                                    if cls[j][i] == GEN:
                                        nc.vector.tensor_add(
                                            ps_s[:], ps_s[:], bias_sb[(j, i)][:])
                                    pt = ptp.tile([KT, QC], MMDT, tag="pt", name="pt")
                                    nc.scalar.activation(pt[:], ps_s[:], EXP)
                                    first, last = idx == 0, idx == len(live) - 1
                                    nc.tensor.matmul(
                                        ps_o[:], lhsT=(v_sb[:, i, :]),
                                        rhs=(pt[:]), start=first, stop=last)
                                    nc.tensor.matmul(
                                        ps_den[:], lhsT=(ones_col[:]),
                                        rhs=(pt[:]), start=first, stop=last)
                                inv_sb = ptp.tile([1, QC], F32, tag="inv", name="inv")
                                nc.vector.reciprocal(inv_sb[:], ps_den[:])
                                ps_b = bpp.tile([KT, QC], F32, tag="b", name="b")
                                nc.tensor.matmul(ps_b[:], lhsT=ones_row[:],
                                                 rhs=inv_sb[:], start=True, stop=True)
                                invb = ptp.tile([KT, QC], F32, tag="invb", name="invb")
                                nc.scalar.copy(invb[:], ps_b[:])
                                nc.vector.tensor_mul(
                                    attn_sb[:, h, j * QC:(j + 1) * QC],
                                    ps_o[:], invb[:])
                                if DEBUG_DUMP:
                                    nc.sync.dma_start(
                                        dbg_attn[:, h, j * QC:(j + 1) * QC],
                                        attn_sb[:, h, j * QC:(j + 1) * QC])

                    # ---------------- Phase C: O-projection -------------
                    with tc.tile_pool(name="wo", bufs=1) as wop, \
                         tc.tile_pool(name="ost", bufs=4) as osp, \
                         tc.tile_pool(name="cps", bufs=4, space="PSUM") as cpp:
                        wo_sb = wop.tile([KT, HPC, S], MMDT, tag="wo", name="wo")
                        for h in range(HPC):
                            nc.sync.dma_start(wo_sb[:, h, :], wo_d[h])
                        for mt in range(NKT):
                            for nck in range(NQ):
                                ps = cpp.tile([KT, QC], F32, tag="c", name="c")
                                for h in range(HPC):
                                    nc.tensor.matmul(
                                        ps[:],
                                        lhsT=(attn_sb[:, h, mt * KT:(mt + 1) * KT]),
                                        rhs=(wo_sb[:, h, nck * QC:(nck + 1) * QC]),
                                        start=(h == 0), stop=(h == HPC - 1))
                                ost = osp.tile([KT, QC], F32, tag="ost", name="ost")
                                nc.scalar.copy(ost[:], ps[:])
                                nc.sync.dma_start(
                                    out_d[mt * KT:(mt + 1) * KT,
                                          nck * QC:(nck + 1) * QC], ost[:])

    nc.compile()
    res = bass_utils.run_bass_kernel_spmd(
        nc, in_maps, core_ids=list(range(N_CORES)), trace=True)
    return res


def kernel(hidden_states, masks, attn_bias, cos, sin, wq, wk, wv, wo,
           position_ids):
    global LAST_EXEC_TIME_NS, LAST_RESULTS
    hidden_states = np.asarray(hidden_states, np.float32)
    masks = np.asarray(masks, np.float32)
    attn_bias = np.asarray(attn_bias, np.float32)
    cos = np.asarray(cos, np.float32)
    sin = np.asarray(sin, np.float32)
    wq, wk, wv, wo = (np.asarray(w, np.float32) for w in (wq, wk, wv, wo))
    position_ids = np.asarray(position_ids)

    combined = attn_bias[:, 0] + masks          # [B, S, S]
    cls = _classify(combined)

    # Safety for the skipped softmax max-subtraction: every row must keep at
    # least one tile whose bias cannot underflow exp() (|logit| is O(10)).
    for b in range(B):
        for j in range(NQ):
            live_cols = [i for i in range(NKT) if cls[j][i] != SKIP]
            block = combined[b, j * QC:(j + 1) * QC][:,
                    [c for i in live_cols for c in range(i * KT, (i + 1) * KT)]]
            if block.max(axis=1).min() < -1e4:
                raise NotImplementedError(
                    "bias pattern leaves a fully-suppressed row; "
                    "max-free softmax unsafe")

    gen_tiles = [(j, i) for j in range(NQ) for i in range(NKT)
                 if cls[j][i] == GEN]
    n_gen = len(gen_tiles)

    inv_sqrt_hd = 1.0 / math.sqrt(HD)

    in_maps = []
    for core in range(N_CORES):
        b, hq = divmod(core, HPC)
        heads = range(hq * HPC, hq * HPC + HPC)

        xT = np.ascontiguousarray(hidden_states[b].T).reshape(NKT, KT, S)

        wq_c = np.stack([np.ascontiguousarray(
            wq[h * HD:(h + 1) * HD, :].T).reshape(NKT, KT, HD) for h in heads])
        wk_c = np.stack([np.ascontiguousarray(
            wk[h * HD:(h + 1) * HD, :].T).reshape(NKT, KT, HD) for h in heads])
        wv_c = np.ascontiguousarray(
            wv[hq * DPC:(hq + 1) * DPC, :].T).reshape(NKT, KT, DPC)
        wo_c = np.ascontiguousarray(
            wo[:, hq * DPC:(hq + 1) * DPC].T).reshape(HPC, KT, S)

        cos_g = cos[position_ids[b]]            # [S, HD]
        sin_g = sin[position_ids[b]]
        cosT = np.ascontiguousarray(cos_g.T)    # [HD, S]
        sinT = np.ascontiguousarray(sin_g.T)
        sinm = np.concatenate([-sinT[:HD // 2], sinT[HD // 2:]], axis=0)

        m = {
            "ones": np.ones((KT, 1), np.float32),
            "xT": xT,
            "wq": wq_c, "wk": wk_c, "wv": wv_c, "woT": wo_c,
            "cos_q": (cosT * inv_sqrt_hd).astype(np.float32),
            "sinm_q": (sinm * inv_sqrt_hd).astype(np.float32),
            "cos_k": cosT.astype(np.float32),
            "sinm_k": sinm.astype(np.float32),
        }
        if n_gen:
            m["bias_gen"] = np.stack([
                np.ascontiguousarray(combined[b, j * QC:(j + 1) * QC,
                                     i * KT:(i + 1) * KT].T)
                for (j, i) in gen_tiles])
        in_maps.append(m)

    res = _build_and_run(in_maps, cls, n_gen)
    LAST_EXEC_TIME_NS = res.exec_time_ns
    LAST_RESULTS = res

    out = np.zeros((B, S, H), np.float32)
    for core in range(N_CORES):
        b = core // HPC
        out[b] += res.results[core]["out"]
    return out


# revision 10
# speedup vs baseline: 1.0226x; 1.0226x over previous
"""Trainium2 Bass kernel for nn_Attention (B=2, S=2048, H=2048, NH=16, HD=128).

Sharding: 2-way batch DP x 4-way head TP -> 8 NeuronCores.
Core c = b*4 + hq handles batch b, heads [4*hq, 4*hq+4).

Per-core pipeline (all fp32 storage, matmuls optionally float32r mode):
  Phase A: Q/K/V projections from resident x^T; RoPE fused into the PSUM
           evacuation of Q/K; q^T/k^T/v spilled to DRAM scratch.
  Phase B: per (head, q-chunk of 512): scores computed TRANSPOSED
           (S^T[k,q] = K^T stationary x Q^T moving), additive bias tiles only
           where the host-side classification says they are needed, exp on
           ACT (PSUM->SBUF), denominator via ones-matmul PSUM accumulation,
           PV accumulation with V stationary, normalization by broadcast
           reciprocal fused into the PV PSUM evacuation.
  Phase C: partial O-projection (contraction over this core's 512 attention
           features) -> out_partial [2048, 2048].
Host sums the 4 head-group partials per batch.

Causal masking is exploited structurally: host classifies each
(q-chunk 512, k-tile 128) tile of (attn_bias + masks) as SKIP (all <= -1e8),
ZERO (all == 0) or GENERAL (bias data streamed + added). Fully-masked score
entries in GENERAL tiles underflow exp() to exactly 0.0, matching the
reference's softmax on -1e9-masked logits. Softmax max-subtraction is skipped:
logits here are O(10) so exp() cannot overflow, and the host verifies every
row has at least one live (not strongly negative) tile.
"""
import math
import sys

sys.path.insert(0, '/opt/trn_rl_repo')

import numpy as np

B, S, H, NH, HD = 2, 2048, 2048, 16, 128
N_CORES = 8
HPC = 4               # heads per core
QC = 512              # q-chunk (matmul moving free dim)
KT = 128              # k-tile (PE contraction dim)
NQ = S // QC          # 4
NKT = S // KT         # 16
DPC = HPC * HD        # 512 features per core

SKIP, ZERO, GEN = 0, 1, 2

# matmul dtype knobs: float32r streams fp32 through the PE in single-pass
# mode (4x faster at free dim >= 256) at reduced internal precision.
USE_F32R = True
DEBUG_DUMP = False

LAST_EXEC_TIME_NS = None
LAST_RESULTS = None


def _classify(combined):
    """combined: [B, S, S] additive bias (attn_bias + masks), b-th batch.
    Returns cls[NQ][NKT] merged over batches, and per-batch GEN tile data."""
    cls = np.full((NQ, NKT), ZERO, np.int32)
    per_b = np.zeros((B, NQ, NKT), np.int32)
    for j in range(NQ):
        for i in range(NKT):
            for b in range(B):
                t = combined[b, j * QC:(j + 1) * QC, i * KT:(i + 1) * KT]
                if t.max() <= -1e8:
                    per_b[b, j, i] = SKIP
                elif not t.any():
                    per_b[b, j, i] = ZERO
                else:
                    per_b[b, j, i] = GEN
    for j in range(NQ):
        for i in range(NKT):
            kinds = set(per_b[:, j, i])
            if kinds == {SKIP}:
                cls[j, i] = SKIP
            elif kinds == {ZERO}:
                cls[j, i] = ZERO
            else:
                cls[j, i] = GEN
    return cls


def _build(cls, n_gen):
    import concourse.bacc as bacc
    import concourse.mybir as mybir
    import concourse.tile as tile

    F32 = mybir.dt.float32
    F32R = mybir.dt.float32r
    EXP = mybir.ActivationFunctionType.Exp

    MMDT = F32R if USE_F32R else F32

    nc = bacc.Bacc("TRN2", target_bir_lowering=False, debug=False,
                   num_devices=N_CORES)

    xT_d = nc.dram_tensor("xT", [NKT, KT, S], MMDT, kind="ExternalInput").ap()
    wq_d = nc.dram_tensor("wq", [HPC, NKT, KT, HD], MMDT, kind="ExternalInput").ap()
    wk_d = nc.dram_tensor("wk", [HPC, NKT, KT, HD], MMDT, kind="ExternalInput").ap()
    wv_d = nc.dram_tensor("wv", [NKT, KT, DPC], MMDT, kind="ExternalInput").ap()
    wo_d = nc.dram_tensor("woT", [HPC, KT, S], MMDT, kind="ExternalInput").ap()
    cq_d = nc.dram_tensor("cos_q", [HD, S], F32, kind="ExternalInput").ap()
    sq_d = nc.dram_tensor("sinm_q", [HD, S], F32, kind="ExternalInput").ap()
    ck_d = nc.dram_tensor("cos_k", [HD, S], F32, kind="ExternalInput").ap()
    sk_d = nc.dram_tensor("sinm_k", [HD, S], F32, kind="ExternalInput").ap()
    if n_gen:
        bg_d = nc.dram_tensor("bias_gen", [n_gen, KT, QC], F32,
                              kind="ExternalInput").ap()
    ones_d = nc.dram_tensor("ones", [KT, 1], MMDT, kind="ExternalInput").ap()
    onesr_d = nc.dram_tensor("ones_row", [1, KT], MMDT, kind="ExternalInput").ap()
    out_d = nc.dram_tensor("out", [S, S], F32, kind="ExternalOutput").ap()
    if DEBUG_DUMP:
        dbg_q = nc.dram_tensor("dbg_q", [HPC, HD, S], MMDT, kind="ExternalOutput").ap()
        dbg_k = nc.dram_tensor("dbg_k", [HPC, HD, S], MMDT, kind="ExternalOutput").ap()
        dbg_v = nc.dram_tensor("dbg_v", [NKT, KT, DPC], MMDT, kind="ExternalOutput").ap()
        dbg_attn = nc.dram_tensor("dbg_attn", [HD, HPC, S], MMDT, kind="ExternalOutput").ap()

    with tile.TileContext(nc) as tc:
        with tc.tile_pool(name="dram", bufs=1, space="DRAM") as dpool:
            qts = [dpool.tile([HD, S], MMDT, tag=f"qts{h}", name=f"qts{h}")
                   for h in range(HPC)]
            kts_d = [dpool.tile([HD, S], MMDT, tag=f"kts{h}", name=f"kts{h}")
                     for h in range(HPC)]
            vts = dpool.tile([NKT, KT, DPC], MMDT, tag="vts", name="vts")

            with tc.tile_pool(name="small", bufs=1) as spool:
                ones_col = spool.tile([KT, 1], MMDT, name="ones_col")
                nc.sync.dma_start(ones_col[:], ones_d[:])
                ones_row = spool.tile([1, KT], MMDT, name="ones_row")
                nc.sync.dma_start(ones_row[:], onesr_d[:])

                # ---------------- Phase A: projections + RoPE ----------
                with tc.tile_pool(name="xp", bufs=1) as xp:
                    x_sb = [xp.tile([KT, S], MMDT, tag=f"x{kt}", name=f"x{kt}")
                            for kt in range(NKT)]
                    for kt in range(NKT):
                        nc.sync.dma_start(x_sb[kt][:], xT_d[kt])
                    rope_sb = {}
                    for nm, td in (("cq", cq_d), ("sq", sq_d),
                                   ("ck", ck_d), ("sk", sk_d)):
                        t = xp.tile([HD, S], F32, tag=nm, name=nm)
                        nc.sync.dma_start(t[:], td[:])
                        rope_sb[nm] = t

                    # V projection first (frees its PSUM banks early so
                    # phase B can overlap the Q/K passes)
                    with tc.tile_pool(name="vw", bufs=2) as vwp, \
                         tc.tile_pool(name="vst", bufs=3) as vsp, \
                         tc.tile_pool(name="vps", bufs=1, space="PSUM") as vpp:
                        for mtg in range(2):
                            pss = [vpp.tile([KT, DPC], F32, tag=f"vps{m}",
                                            name=f"vps{m}") for m in range(8)]
                            for kt in range(NKT):
                                wv_sb = vwp.tile([KT, DPC], MMDT, tag="wv",
                                                 name="wv")
                                nc.sync.dma_start(wv_sb[:], wv_d[kt])
                                for m in range(8):
                                    mt = mtg * 8 + m
                                    nc.tensor.matmul(
                                        pss[m][:],
                                        lhsT=x_sb[kt][:, mt * KT:(mt + 1) * KT],
                                        rhs=wv_sb[:],
                                        start=(kt == 0), stop=(kt == NKT - 1))
                            for m in range(8):
                                mt = mtg * 8 + m
                                vst = vsp.tile([KT, DPC], MMDT, tag="vst",
                                               name="vst")
                                nc.scalar.copy(vst[:], pss[m][:])
                                nc.sync.dma_start(vts[mt], vst[:])
                                if DEBUG_DUMP:
                                    nc.sync.dma_start(dbg_v[mt], vst[:])

                    # Q and K per head, interleaved, so phase B's head 0
                    # inputs are ready early
                    with tc.tile_pool(name="qk", bufs=2) as qkp, \
                         tc.tile_pool(name="qkps", bufs=2, space="PSUM") as pp:
                        for h in range(HPC):
                            for (w_d, cn, sn, dsts, dbgt) in (
                                    (wq_d, "cq", "sq", qts, "q"),
                                    (wk_d, "ck", "sk", kts_d, "k")):
                                cos_sb, sin_sb = rope_sb[cn], rope_sb[sn]
                                w_sb = qkp.tile([KT, NKT, HD], MMDT, tag="w",
                                                name="w")
                                for kt in range(NKT):
                                    nc.sync.dma_start(w_sb[:, kt, :], w_d[h, kt])
                                for sc in range(NQ):
                                    ps = pp.tile([KT, QC], F32, tag="ps",
                                                 name="ps")
                                    for kt in range(NKT):
                                        nc.tensor.matmul(
                                            ps[:],
                                            lhsT=w_sb[:, kt, :],
                                            rhs=x_sb[kt][:, sc * QC:(sc + 1) * QC],
                                            start=(kt == 0), stop=(kt == NKT - 1))
                                    st = qkp.tile([KT, QC], F32, tag="st",
                                                  name="st")
                                    sw = qkp.tile([KT, QC], F32, tag="sw",
                                                  name="sw")
                                    str_ = qkp.tile([KT, QC], MMDT, tag="str",
                                                    name="str")
                                    csl = slice(sc * QC, (sc + 1) * QC)
                                    # rotate-half via partition-offset reads
                                    nc.vector.tensor_mul(
                                        sw[0:64, :], ps[64:128, :],
                                        sin_sb[0:64, csl])
                                    nc.vector.tensor_mul(
                                        sw[64:128, :], ps[0:64, :],
                                        sin_sb[64:128, csl])
                                    nc.vector.tensor_mul(st[:], ps[:],
                                                         cos_sb[:, csl])
                                    nc.vector.tensor_add(str_[:], st[:], sw[:])
                                    nc.sync.dma_start(dsts[h][:, csl], str_[:])
                                    if DEBUG_DUMP:
                                        dbg = dbg_q if dbgt == "q" else dbg_k
                                        nc.sync.dma_start(dbg[h][:, csl], str_[:])

                # ---------------- Phase B: attention --------------------
                with tc.tile_pool(name="attn", bufs=1) as ap_pool:
                    attn_sb = ap_pool.tile([HD, HPC, S], MMDT, tag="attn",
                                           name="attn")

                    with tc.tile_pool(name="bt", bufs=1) as btp, \
                         tc.tile_pool(name="kv", bufs=2) as kvp, \
                         tc.tile_pool(name="pt", bufs=4) as ptp, \
                         tc.tile_pool(name="sps", bufs=2, space="PSUM") as spp, \
                         tc.tile_pool(name="ops", bufs=2, space="PSUM") as opp, \
                         tc.tile_pool(name="dps", bufs=1, space="PSUM") as dpp, \
                         tc.tile_pool(name="bps", bufs=1, space="PSUM") as bpp:

                        bias_sb = {}
                        gi = 0
                        for j in range(NQ):
                            for i in range(NKT):
                                if cls[j][i] == GEN:
                                    t = btp.tile([KT, QC], F32, tag=f"bias{gi}",
                                                 name=f"bias{gi}")
                                    nc.sync.dma_start(t[:], bg_d[gi])
                                    bias_sb[(j, i)] = t
                                    gi += 1

                        for h in range(HPC):
                            k_sb = kvp.tile([HD, S], MMDT, tag="k", name="k")
                            nc.sync.dma_start(k_sb[:], kts_d[h][:])
                            v_sb = kvp.tile([KT, NKT, HD], MMDT, tag="v",
                                            name="v")
                            for kt in range(NKT):
                                nc.sync.dma_start(
                                    v_sb[:, kt, :],
                                    vts[kt][:, h * HD:(h + 1) * HD])
                            for j in range(NQ):
                                live = [i for i in range(NKT)
                                        if cls[j][i] != SKIP]
                                q_sb = kvp.tile([HD, QC], MMDT, tag="q",
                                                name="q")
                                nc.sync.dma_start(
                                    q_sb[:], qts[h][:, j * QC:(j + 1) * QC])
                                ps_o = opp.tile([HD, QC], F32, tag="o",
                                                name="o")
                                ps_den = dpp.tile([1, QC], F32, tag="den",
                                                  name="den")
                                for idx, i in enumerate(live):
                                    ps_s = spp.tile([KT, QC], F32, tag="s",
                                                    name="s")
                                    nc.tensor.matmul(
                                        ps_s[:],
                                        lhsT=k_sb[:, i * KT:(i + 1) * KT],
                                        rhs=q_sb[:], start=True, stop=True)
                                    if cls[j][i] == GEN:
                                        nc.vector.tensor_add(
                                            ps_s[:], ps_s[:],
                                            bias_sb[(j, i)][:])
                                    pt = ptp.tile([KT, QC], MMDT, tag="pt",
                                                  name="pt")
                                    nc.scalar.activation(pt[:], ps_s[:], EXP)
                                    first = idx == 0
                                    last = idx == len(live) - 1
                                    nc.tensor.matmul(
                                        ps_o[:], lhsT=v_sb[:, i, :],
                                        rhs=pt[:], start=first, stop=last)
                                    nc.tensor.matmul(
                                        ps_den[:], lhsT=ones_col[:],
                                        rhs=pt[:], start=first, stop=last)
                                # move the denominator off PSUM fast, then
                                # normalize: inv -> broadcast -> scale
                                den_sb = ptp.tile([1, QC], F32, tag="den_sb",
                                                  name="den_sb")
                                nc.scalar.copy(den_sb[:], ps_den[:])
                                inv_sb = ptp.tile([1, QC], MMDT, tag="inv",
                                                  name="inv")
                                with nc.allow_low_precision(
                                        reason="f32r feeds broadcast matmul"):
                                    nc.vector.reciprocal(inv_sb[:], den_sb[:])
                                ps_b = bpp.tile([KT, QC], F32, tag="b",
                                                name="b")
                                nc.tensor.matmul(ps_b[:], lhsT=ones_row[:],
                                                 rhs=inv_sb[:], start=True,
                                                 stop=True)
                                invb = ptp.tile([KT, QC], F32, tag="invb",
                                                name="invb")
                                nc.scalar.copy(invb[:], ps_b[:])
                                nc.vector.tensor_mul(
                                    attn_sb[:, h, j * QC:(j + 1) * QC],
                                    ps_o[:], invb[:])
                                if DEBUG_DUMP:
                                    nc.sync.dma_start(
                                        dbg_attn[:, h, j * QC:(j + 1) * QC],
                                        attn_sb[:, h, j * QC:(j + 1) * QC])

                    # ---------------- Phase C: O-projection -------------
                    with tc.tile_pool(name="wo", bufs=1) as wop, \
                         tc.tile_pool(name="ost", bufs=4) as osp, \
                         tc.tile_pool(name="cps", bufs=4, space="PSUM") as cpp:
                        wo_sb = wop.tile([KT, HPC, S], MMDT, tag="wo",
                                         name="wo")
                        for h in range(HPC):
                            nc.sync.dma_start(wo_sb[:, h, :], wo_d[h])
                        for mt in range(NKT):
                            for nck in range(NQ):
                                ps = cpp.tile([KT, QC], F32, tag="c", name="c")
                                for h in range(HPC):
                                    nc.tensor.matmul(
                                        ps[:],
                                        lhsT=attn_sb[:, h,
                                                     mt * KT:(mt + 1) * KT],
                                        rhs=wo_sb[:, h,
                                                  nck * QC:(nck + 1) * QC],
                                        start=(h == 0), stop=(h == HPC - 1))
                                ost = osp.tile([KT, QC], F32, tag="ost",
                                               name="ost")
                                nc.scalar.copy(ost[:], ps[:])
                                nc.sync.dma_start(
                                    out_d[mt * KT:(mt + 1) * KT,
                                          nck * QC:(nck + 1) * QC], ost[:])

    nc.compile()
    return nc


def _build_and_run(in_maps, cls, n_gen):
    from concourse import bass_utils
    try:
        sys.path.insert(0, '/root/problem')
        import axon_profile_patch
        axon_profile_patch.install()
    except Exception:
        pass
    nc = _build(cls, n_gen)
    res = bass_utils.run_bass_kernel_spmd(
        nc, in_maps, core_ids=list(range(N_CORES)), trace=True)
    return res


def kernel(hidden_states, masks, attn_bias, cos, sin, wq, wk, wv, wo,
           position_ids):
    global LAST_EXEC_TIME_NS, LAST_RESULTS
    hidden_states = np.asarray(hidden_states, np.float32)
    masks = np.asarray(masks, np.float32)
    attn_bias = np.asarray(attn_bias, np.float32)
    cos = np.asarray(cos, np.float32)
    sin = np.asarray(sin, np.float32)
    wq, wk, wv, wo = (np.asarray(w, np.float32) for w in (wq, wk, wv, wo))
    position_ids = np.asarray(position_ids)

    combined = attn_bias[:, 0] + masks          # [B, S, S]
    cls = _classify(combined)

    # Safety for the skipped softmax max-subtraction: every row must keep at
    # least one tile whose bias cannot underflow exp() (|logit| is O(10)).
    for b in range(B):
        for j in range(NQ):
            live_cols = [i for i in range(NKT) if cls[j][i] != SKIP]
            block = combined[b, j * QC:(j + 1) * QC][:,
                    [c for i in live_cols for c in range(i * KT, (i + 1) * KT)]]
            if block.max(axis=1).min() < -1e4:
                raise NotImplementedError(
                    "bias pattern leaves a fully-suppressed row; "
                    "max-free softmax unsafe")

    gen_tiles = [(j, i) for j in range(NQ) for i in range(NKT)
                 if cls[j][i] == GEN]
    n_gen = len(gen_tiles)

    inv_sqrt_hd = 1.0 / math.sqrt(HD)

    in_maps = []
    for core in range(N_CORES):
        b, hq = divmod(core, HPC)
        heads = range(hq * HPC, hq * HPC + HPC)

        xT = np.ascontiguousarray(hidden_states[b].T).reshape(NKT, KT, S)

        wq_c = np.stack([np.ascontiguousarray(
            wq[h * HD:(h + 1) * HD, :].T).reshape(NKT, KT, HD) for h in heads])
        wk_c = np.stack([np.ascontiguousarray(
            wk[h * HD:(h + 1) * HD, :].T).reshape(NKT, KT, HD) for h in heads])
        wv_c = np.ascontiguousarray(
            wv[hq * DPC:(hq + 1) * DPC, :].T).reshape(NKT, KT, DPC)
        wo_c = np.ascontiguousarray(
            wo[:, hq * DPC:(hq + 1) * DPC].T).reshape(HPC, KT, S)

        cos_g = cos[position_ids[b]]            # [S, HD]
        sin_g = sin[position_ids[b]]
        cosT = np.ascontiguousarray(cos_g.T)    # [HD, S]
        sinT = np.ascontiguousarray(sin_g.T)
        sinm = np.concatenate([-sinT[:HD // 2], sinT[HD // 2:]], axis=0)

        m = {
            "ones": np.ones((KT, 1), np.float32),
            "ones_row": np.ones((1, KT), np.float32),
            "xT": xT,
            "wq": wq_c, "wk": wk_c, "wv": wv_c, "woT": wo_c,
            "cos_q": (cosT * inv_sqrt_hd).astype(np.float32),
            "sinm_q": (sinm * inv_sqrt_hd).astype(np.float32),
            "cos_k": cosT.astype(np.float32),
            "sinm_k": sinm.astype(np.float32),
        }
        if n_gen:
            m["bias_gen"] = np.stack([
                np.ascontiguousarray(combined[b, j * QC:(j + 1) * QC,
                                     i * KT:(i + 1) * KT].T)
                for (j, i) in gen_tiles])
        in_maps.append(m)

    res = _build_and_run(in_maps, cls, n_gen)
    LAST_EXEC_TIME_NS = res.exec_time_ns
    LAST_RESULTS = res

    out = np.zeros((B, S, H), np.float32)
    for core in range(N_CORES):
        b = core // HPC
        out[b] += res.results[core]["out"]
    return out
